# revision 28
# baseline (speedup 1.0000x reference)
"""Multi-head attention (B=2, S=2048, D=1024, H=16) on 8 Trainium2 NeuronCores.

Sharding: core c handles batch b = c//4 and head group g = c%4 (4 heads, 256
of the 1024 model dims). Per core:
  qT/kT = (X @ W_{Q,K}[:, g])^T  [256, 2048]   (computed transposed; score
                                                scale 1/sqrt(dk) folded into
                                                W_Q and b_Q on host)
  v     =  X @ W_V[:, g]         [2048, 256]   (natural layout, augmented with
                                                a ones column per head)
  s^T   = k q^T per head         [2048, 2048]  (keys on partitions so both
                                                attention matmuls contract on
                                                the partition dim; no
                                                transposes anywhere)
  exp on ScalarE; the M=65 attn@v matmul against [v | 1] yields both the
  attention output rows (0:64) and the softmax denominator (row 64).
  out^T[e, q] is scaled by 1/denom via a K=2 selector-matmul broadcast, then
  the partial output projection out^T @ W_O[g-rows, :].
Partials are summed across the 4 cores of a batch with chunked ReduceScatter;
the core with g==0 contributes b_O (b_O input is zeroed elsewhere).

All matmuls run as float32r (full-rate PE; ~1e-3 rel err vs fp32).
"""

import sys

if "/opt/trn_rl_repo" not in sys.path:
    sys.path.insert(0, "/opt/trn_rl_repo")

import ml_dtypes
import numpy as np

import concourse.bass as bass
import concourse.mybir as mybir
import concourse.tile as tile
from concourse import bacc
from concourse.bass_utils import run_bass_kernel_spmd

B, S, D = 2, 2048, 1024
H, DK = 16, 64
N_CORES = 8
HPC = 4  # heads per core
EC = HPC * DK  # 256 local model dims per core
GROUPS = [[0, 1, 2, 3], [4, 5, 6, 7]]
F32 = mybir.dt.float32
F32R = mybir.dt.float32r
BF16 = mybir.dt.bfloat16
ATT_DT = BF16  # dtype for scores/av matmul operands

NJ = 2  # q-chunks of 1024
JW = S // NJ
NI = S // 128  # k-tiles
NP = HPC // 2  # head pairs

# q-column permutation: perm-block r (256 wide) of chunk j = global rows
# [r*512 + j*256 : r*512 + (j+1)*256], so A2A slot r always carries the rows
# core r outputs, half per j-chunk.
_PERM = np.concatenate(
    [np.arange(r * 512 + j * 256, r * 512 + (j + 1) * 256) for j in range(2) for r in range(4)]
)


def _wlayout(w):
    """[1024, EC] -> [128, 8, EC] matching the SBUF lhsT tile layout."""
    return np.ascontiguousarray(w.reshape(8, 128, EC).transpose(1, 0, 2))


def _wo_stack(W_O, b):
    """[128, 16, D]: e-chunk rows for A2A slots 0..8 (W_O rows for same-batch
    slots, zeros for the other batch), pre-arranged for SBUF."""
    stk = np.zeros((N_CORES * EC, D), np.float32)
    gs = slice(b * 4 * EC, (b + 1) * 4 * EC)
    stk[gs] = W_O
    out = stk.reshape(16, 128, D).transpose(1, 0, 2)
    return np.ascontiguousarray(out).astype(ml_dtypes.bfloat16)


# K=2 selector: col block 0 broadcasts recip row 0, block 1 broadcasts row 1.
_SELC = np.zeros((2, 128), np.float32)
_SELC[0, 0:64] = 1.0
_SELC[1, 64:128] = 1.0


def _build_nc():
    nc = bacc.Bacc(None, num_devices=N_CORES, num_swdge_queues=4)

    xqt = nc.dram_tensor("xqt", [D, S], F32R, kind="ExternalInput")
    xkt = nc.dram_tensor("xkt", [D, S], F32R, kind="ExternalInput")
    xvt = nc.dram_tensor("xvt", [D, S], F32R, kind="ExternalInput")
    wq = nc.dram_tensor("wq", [128, 8, EC], F32R, kind="ExternalInput")
    wk = nc.dram_tensor("wk", [128, 8, EC], F32R, kind="ExternalInput")
    wv = nc.dram_tensor("wv", [128, 8, EC], F32R, kind="ExternalInput")
    wo = nc.dram_tensor("wo", [128, 16, D], BF16, kind="ExternalInput")
    bq = nc.dram_tensor("bq", [EC], F32, kind="ExternalInput")
    bk = nc.dram_tensor("bk", [EC], F32, kind="ExternalInput")
    bv = nc.dram_tensor("bv", [EC], F32, kind="ExternalInput")
    bo = nc.dram_tensor("bo", [D], F32, kind="ExternalInput")
    selc = nc.dram_tensor("selc", [2, 128], F32R, kind="ExternalInput")

    a2a_in = [
        nc.dram_tensor(f"a2a_in{j}", [N_CORES, EC, 256], BF16) for j in range(NJ)
    ]
    a2a_out = [
        nc.dram_tensor(f"a2a_out{j}", [N_CORES, EC, 256], BF16) for j in range(NJ)
    ]
    out = nc.dram_tensor("out", [NJ, 256, D], F32, kind="ExternalOutput")

    with tile.TileContext(nc) as tc:
        with (
            tc.tile_pool(name="res", bufs=1) as res,
            tc.tile_pool(name="xt", bufs=3) as xt_pool,
            tc.tile_pool(name="exp", bufs=3) as exp_pool,
            tc.tile_pool(name="osb", bufs=3) as osb_pool,
            tc.tile_pool(name="lr", bufs=2) as lr_pool,
            tc.tile_pool(name="a2l", bufs=16) as a2l_pool,
            tc.tile_pool(name="ps", bufs=1, space="PSUM") as ps,
        ):
            # --- weights / constants resident in SBUF ---
            wq_sb = res.tile([128, 8, EC], F32R, tag="wq")
            wk_sb = res.tile([128, 8, EC], F32R, tag="wk")
            wv_sb = res.tile([128, 8, EC], F32R, tag="wv")
            wo_sb = res.tile([128, 16, D], BF16, tag="wo")
            nc.gpsimd.dma_start(out=wk_sb, in_=wk[:])
            nc.gpsimd.dma_start(out=wv_sb, in_=wv[:])
            nc.gpsimd.dma_start(out=wq_sb, in_=wq[:])
            nc.gpsimd.dma_start(out=wo_sb, in_=wo[:])

            bq_sb = res.tile([128, 2], F32, tag="bq")
            bk_sb = res.tile([128, 2], F32, tag="bk")
            nc.gpsimd.dma_start(out=bq_sb, in_=bq[:].rearrange("(c p) -> p c", p=128))
            nc.gpsimd.dma_start(out=bk_sb, in_=bk[:].rearrange("(c p) -> p c", p=128))
            bv_rep = res.tile([128, EC], F32, tag="bv")
            bo_rep = res.tile([128, D], F32, tag="bo")
            nc.gpsimd.dma_start(
                out=bv_rep,
                in_=bass.AP(tensor=bv[:].tensor, offset=0, ap=[[0, 128], [1, EC]]),
            )
            nc.gpsimd.dma_start(
                out=bo_rep,
                in_=bass.AP(tensor=bo[:].tensor, offset=0, ap=[[0, 128], [1, D]]),
            )

            # selAB: K=2 lhsT broadcasting recip row 0 -> out parts 0:64 (col
            # block 0) and row 1 -> same parts via col block 1.
            selAB = res.tile([2, 128], F32R, tag="selAB")
            nc.gpsimd.dma_start(out=selAB, in_=selc[:])

            # --- residents ---
            kt = [res.tile([128, S], ATT_DT, tag=f"kt{c}", name=f"kt{c}") for c in range(2)]
            qt = [res.tile([128, S], ATT_DT, tag=f"qt{c}", name=f"qt{c}") for c in range(2)]
            # v augmented with a ones column per head: attn@v and the softmax
            # denominator come out of one M=65 matmul.
            v_sb = res.tile([128, NI, HPC, DK + 1], ATT_DT, tag="v")
            nc.vector.memset(v_sb[:, :, :, DK : DK + 1], 1.0)
            outTh = [
                res.tile([64, S], BF16, tag=f"outTh{h}", name=f"outTh{h}")
                for h in range(HPC)
            ]

            # --- projections (X streamed once, full-width contiguous DMAs) ---
            # kT / qT: out[e, s] accumulated over d; lhsT = W d-chunk, rhs = X^T.
            for _pname, xsrc, w_sb, b_sb, dst in (
                ("k", xkt, wk_sb, bk_sb, kt),
                ("q", xqt, wq_sb, bq_sb, qt),
            ):
                pk = [
                    ps.tile([128, 1024], F32, tag="q4", bufs=4, name=f"pk{_c}")
                    for _c in range(4)
                ]
                for d in range(8):
                    xtile = xt_pool.tile([128, S], F32R, tag="xt")
                    nc.sync.dma_start(out=xtile, in_=xsrc[d * 128 : (d + 1) * 128, :])
                    for half in range(2):
                        for c in range(2):
                            for n in range(2):
                                nsl = slice(n * 512, (n + 1) * 512)
                                xsl = slice(
                                    half * 1024 + n * 512, half * 1024 + (n + 1) * 512
                                )
                                nc.tensor.matmul(
                                    pk[2 * half + c][:, nsl],
                                    w_sb[:, d, c * 128 : (c + 1) * 128],
                                    xtile[:, xsl],
                                    start=(d == 0),
                                    stop=(d == 7),
                                )
                for half in range(2):
                    for c in range(2):
                        nc.vector.tensor_scalar_add(
                            dst[c][:, half * 1024 : (half + 1) * 1024],
                            pk[2 * half + c],
                            b_sb[:, c : c + 1],
                        )

            # v: natural [s, e]; lhsT = X^T d-chunk m-slice. 4 m-tiles per pass,
            # one [128, 256] accumulator per PSUM bank (2 per q4-slot).
            for q4 in range(4):
                hsl = slice(q4 * 512, (q4 + 1) * 512)
                pvm = [
                    ps.tile([128, 1024], F32, tag="q4", bufs=4, name=f"pv{_m}")
                    for _m in range(2)
                ]
                for d in range(8):
                    xtile = xt_pool.tile([128, S], F32R, tag="xt")
                    nc.sync.dma_start(
                        out=xtile[:, 0:512], in_=xvt[d * 128 : (d + 1) * 128, hsl]
                    )
                    for m in range(4):
                        nc.tensor.matmul(
                            pvm[m // 2][:, (m % 2) * 512 : (m % 2) * 512 + 256],
                            xtile[:, m * 128 : (m + 1) * 128],
                            wv_sb[:, d, :],
                            start=(d == 0),
                            stop=(d == 7),
                        )
                for m in range(4):
                    nc.vector.tensor_add(
                        v_sb[:, q4 * 4 + m, :, 0:DK],
                        pvm[m // 2][
                            :, (m % 2) * 512 : (m % 2) * 512 + 256
                        ].rearrange("p (h d) -> p h d", h=HPC),
                        bv_rep.rearrange("p (h d) -> p h d", h=HPC),
                    )

            # --- attention + output projection, per q-chunk j ---
            for j in range(NJ):
                jsl = slice(j * JW, (j + 1) * JW)
                lrec = [None, None]
                for p in range(NP):
                    hA, hB = 2 * p, 2 * p + 1
                    # av{A,B}: rows 0:64 = attn@v, row 64 = softmax denominator.
                    avA = ps.tile([65, 1024], F32, tag="q4", bufs=4)
                    avB = ps.tile([65, 1024], F32, tag="q4", bufs=4)
                    for i in range(NI):
                        isl = slice(i * 128, (i + 1) * 128)
                        sA = ps.tile([128, 1024], F32, tag="q4", bufs=4)
                        sB = ps.tile([128, 1024], F32, tag="q4", bufs=4)
                        for n in range(2):
                            nsl = slice(n * 512, (n + 1) * 512)
                            qsl = slice(j * JW + n * 512, j * JW + (n + 1) * 512)
                            nc.tensor.matmul(
                                sA[:, nsl],
                                kt[p][0:64, isl],
                                qt[p][0:64, qsl],
                                start=True,
                                stop=True,
                                tile_position=(0, 0),
                            )
                            nc.tensor.matmul(
                                sB[:, nsl],
                                kt[p][64:128, isl],
                                qt[p][64:128, qsl],
                                start=True,
                                stop=True,
                                tile_position=(64, 0),
                            )
                        eA = exp_pool.tile([128, 1024], ATT_DT, tag="exp")
                        eB = exp_pool.tile([128, 1024], ATT_DT, tag="exp")
                        nc.scalar.activation(eA, sA, mybir.ActivationFunctionType.Exp)
                        nc.scalar.activation(eB, sB, mybir.ActivationFunctionType.Exp)
                        for n in range(2):
                            nsl = slice(n * 512, (n + 1) * 512)
                            st = dict(start=(i == 0), stop=(i == NI - 1))
                            nc.tensor.matmul(
                                avA[:, nsl], v_sb[:, i, hA, :], eA[:, nsl], **st
                            )
                            nc.tensor.matmul(
                                avB[:, nsl], v_sb[:, i, hB, :], eB[:, nsl], **st
                            )
                    # drain pair: per-head outT rows; denominators staged out of
                    # partition 64 into lr rows 0/1 via SBUF->SBUF DMA.
                    nc.vector.tensor_copy(outTh[hA][:, jsl], avA[0:64, :])
                    nc.vector.tensor_copy(outTh[hB][:, jsl], avB[0:64, :])
                    stA = lr_pool.tile([65, 1024], F32, tag="stage")
                    stB = lr_pool.tile([65, 1024], F32, tag="stage")
                    nc.vector.tensor_copy(stA[64:65, :], avA[64:65, :])
                    nc.vector.tensor_copy(stB[64:65, :], avB[64:65, :])
                    lr = lr_pool.tile([2, 1024], F32R, tag="lr", name=f"lr{p}")
                    lrec[p] = lr
                    nc.gpsimd.dma_start(out=lr.bitcast(F32)[0:1, :], in_=stA[64:65, :])
                    nc.gpsimd.dma_start(out=lr.bitcast(F32)[1:2, :], in_=stB[64:65, :])
                    with nc.allow_low_precision(
                        reason="denominator recip used as fp32r matmul operand"
                    ):
                        nc.vector.reciprocal(lr, lr)

                # broadcast 1/denom to 64 partitions per head, scale outT
                for p in range(NP):
                    rcP = ps.tile([65, 1024], F32, tag="q4", bufs=4)
                    rcA = ps.tile([65, 1024], F32, tag="q4", bufs=4)
                    for n in range(2):
                        nsl = slice(n * 512, (n + 1) * 512)
                        nc.tensor.matmul(
                            rcP[0:64, nsl],
                            selAB[:, 0:64],
                            lrec[p][:, nsl],
                            start=True,
                            stop=True,
                        )
                        nc.tensor.matmul(
                            rcA[0:64, nsl],
                            selAB[:, 64:128],
                            lrec[p][:, nsl],
                            start=True,
                            stop=True,
                        )
                    nc.vector.tensor_mul(
                        outTh[2 * p][:, jsl], outTh[2 * p][:, jsl], rcP[0:64, :]
                    )
                    nc.vector.tensor_mul(
                        outTh[2 * p + 1][:, jsl], outTh[2 * p + 1][:, jsl], rcA[0:64, :]
                    )

                # ship head outputs: slot r gets our heads for perm-q block r
                # (mirrored to both batch groups; wrong-batch rows of wo are 0)
                for r in range(N_CORES):
                    for h in range(HPC):
                        nc.sync.dma_start(
                            out=a2a_in[j][r, h * DK : (h + 1) * DK, :],
                            in_=outTh[h][
                                :, j * JW + (r % 4) * 256 : j * JW + (r % 4 + 1) * 256
                            ],
                        )
                nc.gpsimd.collective_compute(
                    "AllToAll",
                    mybir.AluOpType.bypass,
                    replica_groups=[list(range(N_CORES))],
                    ins=[a2a_in[j][:]],
                    outs=[a2a_out[j][:]],
                )

            # output projections after all attention: W_O(j0) overlaps A2A(j1)
            for j in range(NJ):
                a2l = []
                for ch in range(16):
                    t = a2l_pool.tile([128, 256], BF16, tag="a2l", name=f"a2l{j}_{ch}")
                    a2l.append(t)
                    nc.sync.dma_start(
                        out=t,
                        in_=a2a_out[j][ch // 2, (ch % 2) * 128 : (ch % 2 + 1) * 128, :],
                    )
                for m in range(2):
                    po = ps.tile([128, 1024], F32, tag="q4", bufs=4)
                    for ch in range(16):
                        for n in range(2):
                            nsl = slice(n * 512, (n + 1) * 512)
                            nc.tensor.matmul(
                                po[:, nsl],
                                a2l[ch][:, m * 128 : (m + 1) * 128],
                                wo_sb[:, ch, nsl],
                                start=(ch == 0),
                                stop=(ch == 15),
                            )
                    ob = osb_pool.tile([128, D], F32, tag="ob")
                    nc.vector.tensor_add(ob, po, bo_rep)
                    nc.sync.dma_start(out=out[j, m * 128 : (m + 1) * 128, :], in_=ob)

    nc.compile()
    return nc


_NC_CACHE = {}


def _get_nc():
    if "nc" not in _NC_CACHE:
        _NC_CACHE["nc"] = _build_nc()
    return _NC_CACHE["nc"]


def kernel(Q, K, V, W_Q, b_Q, W_K, b_K, W_V, b_V, W_O, b_O, _trace=False):
    Q, K, V = (np.asarray(x, np.float32) for x in (Q, K, V))
    W_Q, W_K, W_V, W_O = (np.asarray(x, np.float32) for x in (W_Q, W_K, W_V, W_O))
    b_Q, b_K, b_V, b_O = (np.asarray(x, np.float32) for x in (b_Q, b_K, b_V, b_O))
    scale = np.float32(1.0 / np.sqrt(DK))

    in_maps = []
    for c in range(N_CORES):
        b, g = c // 4, c % 4
        es = slice(g * EC, (g + 1) * EC)
        in_maps.append(
            {
                "xqt": np.ascontiguousarray(Q[b].T[:, _PERM]),
                "xkt": np.ascontiguousarray(K[b].T),
                "xvt": np.ascontiguousarray(V[b].T),
                "wq": _wlayout(W_Q[:, es] * scale),
                "wk": _wlayout(W_K[:, es]),
                "wv": _wlayout(W_V[:, es]),
                "wo": _wo_stack(W_O, b),
                "bq": np.ascontiguousarray(b_Q[es] * scale),
                "bk": np.ascontiguousarray(b_K[es]),
                "bv": np.ascontiguousarray(b_V[es]),
                "bo": b_O,
                "selc": _SELC,
            }
        )

    nc = _get_nc()
    res = run_bass_kernel_spmd(nc, in_maps, list(range(N_CORES)), trace=_trace)

    full = np.empty((B, S, D), np.float32)
    for c in range(N_CORES):
        b, r = c // 4, c % 4
        chunks = res.results[c]["out"]  # [NJ, 256, D]
        full[b, r * 512 : r * 512 + 256, :] = chunks[0]
        full[b, r * 512 + 256 : (r + 1) * 512, :] = chunks[1]
    if _trace:
        return full, res
    return full


# revision 30
# speedup vs baseline: 1.0972x; 1.0972x over previous
"""Multi-head attention (B=2, S=2048, D=1024, H=16) on 8 Trainium2 NeuronCores.

Sharding: core c handles batch b = c//4 and head group g = c%4 (4 heads, 256
of the 1024 model dims). Per core:
  qT/kT = (X @ W_{Q,K}[:, g])^T  [256, 2048]   (computed transposed; score
                                                scale 1/sqrt(dk) folded into
                                                W_Q and b_Q on host)
  v     =  X @ W_V[:, g]         [2048, 256]   (natural layout, augmented with
                                                a ones column per head)
  s^T   = k q^T per head         [2048, 2048]  (keys on partitions so both
                                                attention matmuls contract on
                                                the partition dim; no
                                                transposes anywhere)
  exp on ScalarE; the M=65 attn@v matmul against [v | 1] yields both the
  attention output rows (0:64) and the softmax denominator (row 64).
  out^T[e, q] is scaled by 1/denom via a K=2 selector-matmul broadcast, then
  the partial output projection out^T @ W_O[g-rows, :].
Partials are summed across the 4 cores of a batch with chunked ReduceScatter;
the core with g==0 contributes b_O (b_O input is zeroed elsewhere).

All matmuls run as float32r (full-rate PE; ~1e-3 rel err vs fp32).
"""

import sys

if "/opt/trn_rl_repo" not in sys.path:
    sys.path.insert(0, "/opt/trn_rl_repo")

import ml_dtypes
import numpy as np

import concourse.bass as bass
import concourse.mybir as mybir
import concourse.tile as tile
from concourse import bacc
from concourse.bass_utils import run_bass_kernel_spmd

B, S, D = 2, 2048, 1024
H, DK = 16, 64
N_CORES = 8
HPC = 4  # heads per core
EC = HPC * DK  # 256 local model dims per core
GROUPS = [[0, 1, 2, 3], [4, 5, 6, 7]]
F32 = mybir.dt.float32
F32R = mybir.dt.float32r
BF16 = mybir.dt.bfloat16
ATT_DT = BF16  # dtype for scores/av matmul operands

NJ = 2  # q-chunks of 1024
JW = S // NJ
NI = S // 128  # k-tiles
NP = HPC // 2  # head pairs

# q-column permutation: perm-block r (256 wide) of chunk j = global rows
# [r*512 + j*256 : r*512 + (j+1)*256], so A2A slot r always carries the rows
# core r outputs, half per j-chunk.
_PERM = np.concatenate(
    [np.arange(r * 512 + j * 256, r * 512 + (j + 1) * 256) for j in range(2) for r in range(4)]
)


def _wlayout(w):
    """[1024, EC] -> [128, 8, EC] matching the SBUF lhsT tile layout."""
    return np.ascontiguousarray(w.reshape(8, 128, EC).transpose(1, 0, 2))


def _wo_stack(W_O, b):
    """[128, 16, D]: e-chunk rows for A2A slots 0..8 (W_O rows for same-batch
    slots, zeros for the other batch), pre-arranged for SBUF."""
    stk = np.zeros((N_CORES * EC, D), np.float32)
    gs = slice(b * 4 * EC, (b + 1) * 4 * EC)
    stk[gs] = W_O
    out = stk.reshape(16, 128, D).transpose(1, 0, 2)
    return np.ascontiguousarray(out).astype(ml_dtypes.bfloat16)


# K=2 selector: col block 0 broadcasts recip row 0, block 1 broadcasts row 1.
_SELC = np.zeros((2, 128), np.float32)
_SELC[0, 0:64] = 1.0
_SELC[1, 64:128] = 1.0


def _build_nc():
    nc = bacc.Bacc(None, num_devices=N_CORES, num_swdge_queues=4)

    xqt = nc.dram_tensor("xqt", [D, S], BF16, kind="ExternalInput")
    xkt = nc.dram_tensor("xkt", [D, S], BF16, kind="ExternalInput")
    xvt = nc.dram_tensor("xvt", [D, S], BF16, kind="ExternalInput")
    wq = nc.dram_tensor("wq", [128, 8, EC], BF16, kind="ExternalInput")
    wk = nc.dram_tensor("wk", [128, 8, EC], BF16, kind="ExternalInput")
    wv = nc.dram_tensor("wv", [128, 8, EC], BF16, kind="ExternalInput")
    wo = nc.dram_tensor("wo", [128, 16, D], BF16, kind="ExternalInput")
    bq = nc.dram_tensor("bq", [EC], F32, kind="ExternalInput")
    bk = nc.dram_tensor("bk", [EC], F32, kind="ExternalInput")
    bv = nc.dram_tensor("bv", [EC], F32, kind="ExternalInput")
    bo = nc.dram_tensor("bo", [D], F32, kind="ExternalInput")
    selc = nc.dram_tensor("selc", [2, 128], F32R, kind="ExternalInput")

    a2a_in = [
        nc.dram_tensor(f"a2a_in{j}", [N_CORES, EC, 256], BF16) for j in range(NJ)
    ]
    a2a_out = [
        nc.dram_tensor(f"a2a_out{j}", [N_CORES, EC, 256], BF16) for j in range(NJ)
    ]
    out = nc.dram_tensor("out", [NJ, 256, D], F32, kind="ExternalOutput")

    with tile.TileContext(nc) as tc:
        with (
            tc.tile_pool(name="res", bufs=1) as res,
            tc.tile_pool(name="xt", bufs=3) as xt_pool,
            tc.tile_pool(name="exp", bufs=3) as exp_pool,
            tc.tile_pool(name="osb", bufs=3) as osb_pool,
            tc.tile_pool(name="lr", bufs=2) as lr_pool,
            tc.tile_pool(name="a2l", bufs=16) as a2l_pool,
            tc.tile_pool(name="ps", bufs=1, space="PSUM") as ps,
        ):
            # --- weights / constants resident in SBUF ---
            wq_sb = res.tile([128, 8, EC], BF16, tag="wq")
            wk_sb = res.tile([128, 8, EC], BF16, tag="wk")
            wv_sb = res.tile([128, 8, EC], BF16, tag="wv")
            wo_sb = res.tile([128, 16, D], BF16, tag="wo")
            nc.gpsimd.dma_start(out=wk_sb, in_=wk[:])
            nc.gpsimd.dma_start(out=wv_sb, in_=wv[:])
            nc.gpsimd.dma_start(out=wq_sb, in_=wq[:])
            nc.gpsimd.dma_start(out=wo_sb, in_=wo[:])

            bq_sb = res.tile([128, 2], F32, tag="bq")
            bk_sb = res.tile([128, 2], F32, tag="bk")
            nc.gpsimd.dma_start(out=bq_sb, in_=bq[:].rearrange("(c p) -> p c", p=128))
            nc.gpsimd.dma_start(out=bk_sb, in_=bk[:].rearrange("(c p) -> p c", p=128))
            bv_rep = res.tile([128, EC], F32, tag="bv")
            bo_rep = res.tile([128, D], F32, tag="bo")
            nc.gpsimd.dma_start(
                out=bv_rep,
                in_=bass.AP(tensor=bv[:].tensor, offset=0, ap=[[0, 128], [1, EC]]),
            )
            nc.gpsimd.dma_start(
                out=bo_rep,
                in_=bass.AP(tensor=bo[:].tensor, offset=0, ap=[[0, 128], [1, D]]),
            )

            # selAB: K=2 lhsT broadcasting recip row 0 -> out parts 0:64 (col
            # block 0) and row 1 -> same parts via col block 1.
            selAB = res.tile([2, 128], F32R, tag="selAB")
            nc.gpsimd.dma_start(out=selAB, in_=selc[:])

            # --- residents ---
            kt = [res.tile([128, S], ATT_DT, tag=f"kt{c}", name=f"kt{c}") for c in range(2)]
            qt = [res.tile([128, S], ATT_DT, tag=f"qt{c}", name=f"qt{c}") for c in range(2)]
            # v augmented with a ones column per head: attn@v and the softmax
            # denominator come out of one M=65 matmul.
            v_sb = res.tile([128, NI, HPC, DK + 1], ATT_DT, tag="v")
            nc.vector.memset(v_sb[:, :, :, DK : DK + 1], 1.0)
            outTh = [
                res.tile([64, S], BF16, tag=f"outTh{h}", name=f"outTh{h}")
                for h in range(HPC)
            ]

            # --- projections (X streamed once, full-width contiguous DMAs) ---
            # kT / qT: out[e, s] accumulated over d; lhsT = W d-chunk, rhs = X^T.
            for _pname, xsrc, w_sb, b_sb, dst in (
                ("k", xkt, wk_sb, bk_sb, kt),
                ("q", xqt, wq_sb, bq_sb, qt),
            ):
                pk = [
                    ps.tile([128, 1024], F32, tag="q4", bufs=4, name=f"pk{_c}")
                    for _c in range(4)
                ]
                for d in range(8):
                    xtile = xt_pool.tile([128, S], BF16, tag="xt")
                    nc.sync.dma_start(out=xtile, in_=xsrc[d * 128 : (d + 1) * 128, :])
                    for half in range(2):
                        for c in range(2):
                            for n in range(2):
                                nc.tensor.matmul(
                                    pk[2 * half + c][:, n * 512 : (n + 1) * 512],
                                    w_sb[:, d, c * 128 : (c + 1) * 128],
                                    xtile[
                                        :,
                                        half * 1024 + n * 512 : half * 1024
                                        + (n + 1) * 512,
                                    ],
                                    start=(d == 0),
                                    stop=(d == 7),
                                )
                for half in range(2):
                    for c in range(2):
                        nc.vector.tensor_scalar_add(
                            dst[c][:, half * 1024 : (half + 1) * 1024],
                            pk[2 * half + c],
                            b_sb[:, c : c + 1],
                        )

            # v: natural [s, e]; lhsT = X^T d-chunk m-slice. 4 m-tiles per pass,
            # one [128, 256] accumulator per PSUM bank (2 per q4-slot).
            for q4 in range(4):
                hsl = slice(q4 * 512, (q4 + 1) * 512)
                pvm = [
                    ps.tile([128, 1024], F32, tag="q4", bufs=4, name=f"pv{_m}")
                    for _m in range(2)
                ]
                for d in range(8):
                    xtile = xt_pool.tile([128, S], BF16, tag="xt")
                    nc.sync.dma_start(
                        out=xtile[:, 0:512], in_=xvt[d * 128 : (d + 1) * 128, hsl]
                    )
                    for m in range(4):
                        nc.tensor.matmul(
                            pvm[m // 2][:, (m % 2) * 512 : (m % 2) * 512 + 256],
                            xtile[:, m * 128 : (m + 1) * 128],
                            wv_sb[:, d, :],
                            start=(d == 0),
                            stop=(d == 7),
                        )
                for m in range(4):
                    nc.vector.tensor_add(
                        v_sb[:, q4 * 4 + m, :, 0:DK],
                        pvm[m // 2][
                            :, (m % 2) * 512 : (m % 2) * 512 + 256
                        ].rearrange("p (h d) -> p h d", h=HPC),
                        bv_rep.rearrange("p (h d) -> p h d", h=HPC),
                    )

            # --- attention + output projection, per q-chunk j ---
            for j in range(NJ):
                jsl = slice(j * JW, (j + 1) * JW)
                lrec = [None, None]
                for p in range(NP):
                    hA, hB = 2 * p, 2 * p + 1
                    # av{A,B}: rows 0:64 = attn@v, row 64 = softmax denominator.
                    avA = ps.tile([65, 1024], F32, tag="q4", bufs=4)
                    avB = ps.tile([65, 1024], F32, tag="q4", bufs=4)
                    for i in range(NI):
                        isl = slice(i * 128, (i + 1) * 128)
                        sA = ps.tile([128, 1024], F32, tag="q4", bufs=4)
                        sB = ps.tile([128, 1024], F32, tag="q4", bufs=4)
                        for n in range(2):
                            nsl = slice(n * 512, (n + 1) * 512)
                            qsl = slice(j * JW + n * 512, j * JW + (n + 1) * 512)
                            nc.tensor.matmul(
                                sA[:, nsl], kt[p][0:64, isl], qt[p][0:64, qsl],
                                start=True, stop=True, tile_position=(0, 0),
                            )
                            nc.tensor.matmul(
                                sB[:, nsl], kt[p][64:128, isl], qt[p][64:128, qsl],
                                start=True, stop=True, tile_position=(64, 0),
                            )
                        eA = exp_pool.tile([128, 1024], ATT_DT, tag="exp")
                        eB = exp_pool.tile([128, 1024], ATT_DT, tag="exp")
                        nc.scalar.activation(eA, sA, mybir.ActivationFunctionType.Exp)
                        nc.scalar.activation(eB, sB, mybir.ActivationFunctionType.Exp)
                        for n in range(2):
                            nsl = slice(n * 512, (n + 1) * 512)
                            st = dict(start=(i == 0), stop=(i == NI - 1))
                            nc.tensor.matmul(
                                avA[:, nsl], v_sb[:, i, hA, :], eA[:, nsl], **st
                            )
                            nc.tensor.matmul(
                                avB[:, nsl], v_sb[:, i, hB, :], eB[:, nsl], **st
                            )
                    # drain pair: per-head outT rows; denominators staged out of
                    # partition 64 into lr rows 0/1 via SBUF->SBUF DMA.
                    nc.vector.tensor_copy(outTh[hA][:, jsl], avA[0:64, :])
                    nc.vector.tensor_copy(outTh[hB][:, jsl], avB[0:64, :])
                    stA = lr_pool.tile([65, 1024], F32, tag="stage")
                    stB = lr_pool.tile([65, 1024], F32, tag="stage")
                    nc.vector.tensor_copy(stA[64:65, :], avA[64:65, :])
                    nc.vector.tensor_copy(stB[64:65, :], avB[64:65, :])
                    lr = lr_pool.tile([2, 1024], F32R, tag="lr", name=f"lr{p}")
                    lrec[p] = lr
                    nc.gpsimd.dma_start(out=lr.bitcast(F32)[0:1, :], in_=stA[64:65, :])
                    nc.gpsimd.dma_start(out=lr.bitcast(F32)[1:2, :], in_=stB[64:65, :])
                    with nc.allow_low_precision(
                        reason="denominator recip used as fp32r matmul operand"
                    ):
                        nc.vector.reciprocal(lr, lr)

                # broadcast 1/denom to 64 partitions per head, scale outT
                for p in range(NP):
                    rcP = ps.tile([65, 1024], F32, tag="q4", bufs=4)
                    rcA = ps.tile([65, 1024], F32, tag="q4", bufs=4)
                    for n in range(2):
                        nsl = slice(n * 512, (n + 1) * 512)
                        nc.tensor.matmul(
                            rcP[0:64, nsl],
                            selAB[:, 0:64],
                            lrec[p][:, nsl],
                            start=True,
                            stop=True,
                        )
                        nc.tensor.matmul(
                            rcA[0:64, nsl],
                            selAB[:, 64:128],
                            lrec[p][:, nsl],
                            start=True,
                            stop=True,
                        )
                    nc.vector.tensor_mul(
                        outTh[2 * p][:, jsl], outTh[2 * p][:, jsl], rcP[0:64, :]
                    )
                    nc.vector.tensor_mul(
                        outTh[2 * p + 1][:, jsl], outTh[2 * p + 1][:, jsl], rcA[0:64, :]
                    )

                # ship head outputs: slot r gets our heads for perm-q block r
                # (mirrored to both batch groups; wrong-batch rows of wo are 0)
                for r in range(N_CORES):
                    for h in range(HPC):
                        nc.sync.dma_start(
                            out=a2a_in[j][r, h * DK : (h + 1) * DK, :],
                            in_=outTh[h][
                                :, j * JW + (r % 4) * 256 : j * JW + (r % 4 + 1) * 256
                            ],
                        )
                nc.gpsimd.collective_compute(
                    "AllToAll",
                    mybir.AluOpType.bypass,
                    replica_groups=[list(range(N_CORES))],
                    ins=[a2a_in[j][:]],
                    outs=[a2a_out[j][:]],
                )

            # output projections after all attention: W_O(j0) overlaps A2A(j1)
            for j in range(NJ):
                a2l = []
                for ch in range(16):
                    t = a2l_pool.tile([128, 256], BF16, tag="a2l", name=f"a2l{j}_{ch}")
                    a2l.append(t)
                    nc.sync.dma_start(
                        out=t,
                        in_=a2a_out[j][ch // 2, (ch % 2) * 128 : (ch % 2 + 1) * 128, :],
                    )
                for m in range(2):
                    po = ps.tile([128, 1024], F32, tag="q4", bufs=4)
                    for ch in range(16):
                        for n in range(2):
                            nsl = slice(n * 512, (n + 1) * 512)
                            nc.tensor.matmul(
                                po[:, nsl],
                                a2l[ch][:, m * 128 : (m + 1) * 128],
                                wo_sb[:, ch, nsl],
                                start=(ch == 0),
                                stop=(ch == 15),
                            )
                    ob = osb_pool.tile([128, D], F32, tag="ob")
                    nc.vector.tensor_add(ob, po, bo_rep)
                    nc.sync.dma_start(out=out[j, m * 128 : (m + 1) * 128, :], in_=ob)

    nc.compile()
    return nc


_NC_CACHE = {}


def _get_nc():
    if "nc" not in _NC_CACHE:
        _NC_CACHE["nc"] = _build_nc()
    return _NC_CACHE["nc"]


def kernel(Q, K, V, W_Q, b_Q, W_K, b_K, W_V, b_V, W_O, b_O, _trace=False):
    Q, K, V = (np.asarray(x, np.float32) for x in (Q, K, V))
    W_Q, W_K, W_V, W_O = (np.asarray(x, np.float32) for x in (W_Q, W_K, W_V, W_O))
    b_Q, b_K, b_V, b_O = (np.asarray(x, np.float32) for x in (b_Q, b_K, b_V, b_O))
    scale = np.float32(1.0 / np.sqrt(DK))

    in_maps = []
    for c in range(N_CORES):
        b, g = c // 4, c % 4
        es = slice(g * EC, (g + 1) * EC)
        in_maps.append(
            {
                "xqt": np.ascontiguousarray(Q[b].T[:, _PERM]).astype(ml_dtypes.bfloat16),
                "xkt": np.ascontiguousarray(K[b].T).astype(ml_dtypes.bfloat16),
                "xvt": np.ascontiguousarray(V[b].T).astype(ml_dtypes.bfloat16),
                "wq": _wlayout(W_Q[:, es] * scale).astype(ml_dtypes.bfloat16),
                "wk": _wlayout(W_K[:, es]).astype(ml_dtypes.bfloat16),
                "wv": _wlayout(W_V[:, es]).astype(ml_dtypes.bfloat16),
                "wo": _wo_stack(W_O, b),
                "bq": np.ascontiguousarray(b_Q[es] * scale),
                "bk": np.ascontiguousarray(b_K[es]),
                "bv": np.ascontiguousarray(b_V[es]),
                "bo": b_O,
                "selc": _SELC,
            }
        )

    nc = _get_nc()
    res = run_bass_kernel_spmd(nc, in_maps, list(range(N_CORES)), trace=_trace)

    full = np.empty((B, S, D), np.float32)
    for c in range(N_CORES):
        b, r = c // 4, c % 4
        chunks = res.results[c]["out"]  # [NJ, 256, D]
        full[b, r * 512 : r * 512 + 256, :] = chunks[0]
        full[b, r * 512 + 256 : (r + 1) * 512, :] = chunks[1]
    if _trace:
        return full, res
    return full


# revision 31
# speedup vs baseline: 1.3248x; 1.2074x over previous
"""Multi-head attention (B=2, S=2048, D=1024, H=16) on 8 Trainium2 NeuronCores.

Sharding: core c handles batch b = c//4 and head group g = c%4 (4 heads, 256
of the 1024 model dims). Per core:
  qT/kT = (X @ W_{Q,K}[:, g])^T  [256, 2048]   (computed transposed; score
                                                scale 1/sqrt(dk) folded into
                                                W_Q and b_Q on host)
  v     =  X @ W_V[:, g]         [2048, 256]   (natural layout, augmented with
                                                a ones column per head)
  s^T   = k q^T per head         [2048, 2048]  (keys on partitions so both
                                                attention matmuls contract on
                                                the partition dim; no
                                                transposes anywhere)
  exp on ScalarE; the M=65 attn@v matmul against [v | 1] yields both the
  attention output rows (0:64) and the softmax denominator (row 64).
  out^T[e, q] is scaled by 1/denom via a K=2 selector-matmul broadcast, then
  the partial output projection out^T @ W_O[g-rows, :].
Partials are summed across the 4 cores of a batch with chunked ReduceScatter;
the core with g==0 contributes b_O (b_O input is zeroed elsewhere).

All matmuls run as float32r (full-rate PE; ~1e-3 rel err vs fp32).
"""

import sys

if "/opt/trn_rl_repo" not in sys.path:
    sys.path.insert(0, "/opt/trn_rl_repo")

import ml_dtypes
import numpy as np

import concourse.bass as bass
import concourse.mybir as mybir
import concourse.tile as tile
from concourse import bacc
from concourse.bass_utils import run_bass_kernel_spmd

B, S, D = 2, 2048, 1024
H, DK = 16, 64
N_CORES = 8
HPC = 4  # heads per core
EC = HPC * DK  # 256 local model dims per core
GROUPS = [[0, 1, 2, 3], [4, 5, 6, 7]]
F32 = mybir.dt.float32
F32R = mybir.dt.float32r
BF16 = mybir.dt.bfloat16
ATT_DT = BF16  # dtype for scores/av matmul operands

NJ = 2  # q-chunks of 1024
JW = S // NJ
NI = S // 128  # k-tiles
NP = HPC // 2  # head pairs

# q-column permutation: perm-block r (256 wide) of chunk j = global rows
# [r*512 + j*256 : r*512 + (j+1)*256], so A2A slot r always carries the rows
# core r outputs, half per j-chunk.
_PERM = np.concatenate(
    [np.arange(r * 512 + j * 256, r * 512 + (j + 1) * 256) for j in range(2) for r in range(4)]
)


def _wlayout(w):
    """[1024, EC] -> [128, 8, EC] matching the SBUF lhsT tile layout."""
    return np.ascontiguousarray(w.reshape(8, 128, EC).transpose(1, 0, 2))


def _wo_stack(W_O, b):
    """[128, 16, D]: e-chunk rows for A2A slots 0..8 (W_O rows for same-batch
    slots, zeros for the other batch), pre-arranged for SBUF."""
    stk = np.zeros((N_CORES * EC, D), np.float32)
    gs = slice(b * 4 * EC, (b + 1) * 4 * EC)
    stk[gs] = W_O
    out = stk.reshape(16, 128, D).transpose(1, 0, 2)
    return np.ascontiguousarray(out).astype(ml_dtypes.bfloat16)


# K=2 selector: col block 0 broadcasts recip row 0, block 1 broadcasts row 1.
_SELC = np.zeros((2, 128), np.float32)
_SELC[0, 0:64] = 1.0
_SELC[1, 64:128] = 1.0


def _build_nc():
    nc = bacc.Bacc(None, num_devices=N_CORES, num_swdge_queues=4)

    xqt = nc.dram_tensor("xqt", [D, S], BF16, kind="ExternalInput")
    xkt = nc.dram_tensor("xkt", [D, S], BF16, kind="ExternalInput")
    xvt = nc.dram_tensor("xvt", [D, S], BF16, kind="ExternalInput")
    wq = nc.dram_tensor("wq", [128, 8, EC], BF16, kind="ExternalInput")
    wk = nc.dram_tensor("wk", [128, 8, EC], BF16, kind="ExternalInput")
    wv = nc.dram_tensor("wv", [128, 8, EC], BF16, kind="ExternalInput")
    wo = nc.dram_tensor("wo", [128, 16, D], BF16, kind="ExternalInput")
    bq = nc.dram_tensor("bq", [EC], F32, kind="ExternalInput")
    bk = nc.dram_tensor("bk", [EC], F32, kind="ExternalInput")
    bv = nc.dram_tensor("bv", [EC], F32, kind="ExternalInput")
    bo = nc.dram_tensor("bo", [D], F32, kind="ExternalInput")
    selc = nc.dram_tensor("selc", [2, 128], F32R, kind="ExternalInput")

    a2a_in = [
        nc.dram_tensor(f"a2a_in{j}", [N_CORES, EC, 256], BF16) for j in range(NJ)
    ]
    a2a_out = [
        nc.dram_tensor(f"a2a_out{j}", [N_CORES, EC, 256], BF16) for j in range(NJ)
    ]
    out = nc.dram_tensor("out", [NJ, 256, D], F32, kind="ExternalOutput")

    with tile.TileContext(nc) as tc:
        with (
            tc.tile_pool(name="res", bufs=1) as res,
            tc.tile_pool(name="xt", bufs=3) as xt_pool,
            tc.tile_pool(name="exp", bufs=3) as exp_pool,
            tc.tile_pool(name="osb", bufs=3) as osb_pool,
            tc.tile_pool(name="lr", bufs=2) as lr_pool,
            tc.tile_pool(name="a2l", bufs=16) as a2l_pool,
            tc.tile_pool(name="ps", bufs=1, space="PSUM") as ps,
        ):
            # --- weights / constants resident in SBUF ---
            wq_sb = res.tile([128, 8, EC], BF16, tag="wq")
            wk_sb = res.tile([128, 8, EC], BF16, tag="wk")
            wv_sb = res.tile([128, 8, EC], BF16, tag="wv")
            wo_sb = res.tile([128, 16, D], BF16, tag="wo")
            nc.gpsimd.dma_start(out=wk_sb, in_=wk[:])
            nc.gpsimd.dma_start(out=wv_sb, in_=wv[:])
            nc.gpsimd.dma_start(out=wq_sb, in_=wq[:])
            nc.gpsimd.dma_start(out=wo_sb, in_=wo[:])

            bq_sb = res.tile([128, 2], F32, tag="bq")
            bk_sb = res.tile([128, 2], F32, tag="bk")
            nc.gpsimd.dma_start(out=bq_sb, in_=bq[:].rearrange("(c p) -> p c", p=128))
            nc.gpsimd.dma_start(out=bk_sb, in_=bk[:].rearrange("(c p) -> p c", p=128))
            bv_rep = res.tile([128, EC], F32, tag="bv")
            bo_rep = res.tile([128, D], F32, tag="bo")
            nc.gpsimd.dma_start(
                out=bv_rep,
                in_=bass.AP(tensor=bv[:].tensor, offset=0, ap=[[0, 128], [1, EC]]),
            )
            nc.gpsimd.dma_start(
                out=bo_rep,
                in_=bass.AP(tensor=bo[:].tensor, offset=0, ap=[[0, 128], [1, D]]),
            )

            # selAB: K=2 lhsT broadcasting recip row 0 -> out parts 0:64 (col
            # block 0) and row 1 -> same parts via col block 1.
            selAB = res.tile([2, 128], F32R, tag="selAB")
            nc.gpsimd.dma_start(out=selAB, in_=selc[:])

            # --- residents ---
            kt = [res.tile([128, S], ATT_DT, tag=f"kt{c}", name=f"kt{c}") for c in range(2)]
            # per-head q, zero-padded in the complementary 64 partitions so the
            # scores matmul contracts K=128 (full array)
            qtz = [
                res.tile([128, S], ATT_DT, tag=f"qtz{h}", name=f"qtz{h}")
                for h in range(HPC)
            ]
            for h in range(HPC):
                z = slice(64, 128) if h % 2 == 0 else slice(0, 64)
                nc.vector.memset(qtz[h][z, :], 0.0)
            # v augmented with a ones column per head: attn@v and the softmax
            # denominator come out of one M=65 matmul.
            v_sb = res.tile([128, NI, HPC, 2 * DK], ATT_DT, tag="v")
            nc.vector.memset(v_sb, 0.0)
            nc.vector.memset(v_sb[:, :, :, DK : DK + 1], 1.0)
            outTh = [
                res.tile([64, S], BF16, tag=f"outTh{h}", name=f"outTh{h}")
                for h in range(HPC)
            ]

            # --- projections (X streamed once, full-width contiguous DMAs) ---
            # kT / qT: out[e, s] accumulated over d; lhsT = W d-chunk, rhs = X^T.
            for _pname, xsrc, w_sb, b_sb, dst in (
                ("k", xkt, wk_sb, bk_sb, kt),
                ("q", xqt, wq_sb, bq_sb, None),
            ):
                pk = [
                    ps.tile([128, 1024], F32, tag="q4", bufs=4, name=f"pk{_c}")
                    for _c in range(4)
                ]
                for d in range(8):
                    xtile = xt_pool.tile([128, S], BF16, tag="xt")
                    nc.sync.dma_start(out=xtile, in_=xsrc[d * 128 : (d + 1) * 128, :])
                    for half in range(2):
                        for c in range(2):
                            for n in range(2):
                                nc.tensor.matmul(
                                    pk[2 * half + c][:, n * 512 : (n + 1) * 512],
                                    w_sb[:, d, c * 128 : (c + 1) * 128],
                                    xtile[
                                        :,
                                        half * 1024 + n * 512 : half * 1024
                                        + (n + 1) * 512,
                                    ],
                                    start=(d == 0),
                                    stop=(d == 7),
                                )
                for half in range(2):
                    hs2 = slice(half * 1024, (half + 1) * 1024)
                    for c in range(2):
                        if dst is not None:
                            nc.vector.tensor_scalar_add(
                                dst[c][:, hs2], pk[2 * half + c], b_sb[:, c : c + 1]
                            )
                        else:
                            nc.vector.tensor_scalar_add(
                                qtz[2 * c][0:64, hs2],
                                pk[2 * half + c][0:64, :],
                                b_sb[0:64, c : c + 1],
                            )
                            nc.vector.tensor_scalar_add(
                                qtz[2 * c + 1][64:128, hs2],
                                pk[2 * half + c][64:128, :],
                                b_sb[64:128, c : c + 1],
                            )

            # v: natural [s, e]; lhsT = X^T d-chunk m-slice. 4 m-tiles per pass,
            # one [128, 256] accumulator per PSUM bank (2 per q4-slot).
            for q4 in range(4):
                hsl = slice(q4 * 512, (q4 + 1) * 512)
                pvm = [
                    ps.tile([128, 1024], F32, tag="q4", bufs=4, name=f"pv{_m}")
                    for _m in range(2)
                ]
                for d in range(8):
                    xtile = xt_pool.tile([128, S], BF16, tag="xt")
                    nc.sync.dma_start(
                        out=xtile[:, 0:512], in_=xvt[d * 128 : (d + 1) * 128, hsl]
                    )
                    for m in range(4):
                        nc.tensor.matmul(
                            pvm[m // 2][:, (m % 2) * 512 : (m % 2) * 512 + 256],
                            xtile[:, m * 128 : (m + 1) * 128],
                            wv_sb[:, d, :],
                            start=(d == 0),
                            stop=(d == 7),
                        )
                for m in range(4):
                    nc.vector.tensor_add(
                        v_sb[:, q4 * 4 + m, :, 0:DK],
                        pvm[m // 2][
                            :, (m % 2) * 512 : (m % 2) * 512 + 256
                        ].rearrange("p (h d) -> p h d", h=HPC),
                        bv_rep.rearrange("p (h d) -> p h d", h=HPC),
                    )

            # --- attention + output projection, per q-chunk j ---
            for j in range(NJ):
                jsl = slice(j * JW, (j + 1) * JW)
                lrec = [None, None]
                for p in range(NP):
                    hA, hB = 2 * p, 2 * p + 1
                    # av{A,B}: rows 0:64 = attn@v, row 64 = softmax denominator.
                    avA = ps.tile([128, 1024], F32, tag="q4", bufs=4)
                    avB = ps.tile([128, 1024], F32, tag="q4", bufs=4)
                    for i in range(NI):
                        isl = slice(i * 128, (i + 1) * 128)
                        sA = ps.tile([128, 1024], F32, tag="q4", bufs=4)
                        sB = ps.tile([128, 1024], F32, tag="q4", bufs=4)
                        for n in range(2):
                            nsl = slice(n * 512, (n + 1) * 512)
                            qsl = slice(j * JW + n * 512, j * JW + (n + 1) * 512)
                            nc.tensor.matmul(
                                sA[:, nsl], kt[p][:, isl], qtz[hA][:, qsl],
                                start=True, stop=True,
                            )
                            nc.tensor.matmul(
                                sB[:, nsl], kt[p][:, isl], qtz[hB][:, qsl],
                                start=True, stop=True,
                            )
                        eA = exp_pool.tile([128, 1024], ATT_DT, tag="exp")
                        eB = exp_pool.tile([128, 1024], ATT_DT, tag="exp")
                        nc.scalar.activation(eA, sA, mybir.ActivationFunctionType.Exp)
                        nc.scalar.activation(eB, sB, mybir.ActivationFunctionType.Exp)
                        for n in range(2):
                            nsl = slice(n * 512, (n + 1) * 512)
                            st = dict(start=(i == 0), stop=(i == NI - 1))
                            nc.tensor.matmul(
                                avA[:, nsl], v_sb[:, i, hA, :], eA[:, nsl], **st
                            )
                            nc.tensor.matmul(
                                avB[:, nsl], v_sb[:, i, hB, :], eB[:, nsl], **st
                            )
                    # drain pair: per-head outT rows; denominators staged out of
                    # partition 64 into lr rows 0/1 via SBUF->SBUF DMA.
                    nc.vector.tensor_copy(outTh[hA][:, jsl], avA[0:64, :])
                    nc.vector.tensor_copy(outTh[hB][:, jsl], avB[0:64, :])
                    stA = lr_pool.tile([65, 1024], F32, tag="stage")
                    stB = lr_pool.tile([65, 1024], F32, tag="stage")
                    nc.vector.tensor_copy(stA[64:65, :], avA[64:65, :])
                    nc.vector.tensor_copy(stB[64:65, :], avB[64:65, :])
                    lr = lr_pool.tile([2, 1024], F32R, tag="lr", name=f"lr{p}")
                    lrec[p] = lr
                    nc.gpsimd.dma_start(out=lr.bitcast(F32)[0:1, :], in_=stA[64:65, :])
                    nc.gpsimd.dma_start(out=lr.bitcast(F32)[1:2, :], in_=stB[64:65, :])
                    with nc.allow_low_precision(
                        reason="denominator recip used as fp32r matmul operand"
                    ):
                        nc.vector.reciprocal(lr, lr)

                # broadcast 1/denom to 64 partitions per head, scale outT
                for p in range(NP):
                    rcP = ps.tile([65, 1024], F32, tag="q4", bufs=4)
                    rcA = ps.tile([65, 1024], F32, tag="q4", bufs=4)
                    for n in range(2):
                        nsl = slice(n * 512, (n + 1) * 512)
                        nc.tensor.matmul(
                            rcP[0:64, nsl],
                            selAB[:, 0:64],
                            lrec[p][:, nsl],
                            start=True,
                            stop=True,
                        )
                        nc.tensor.matmul(
                            rcA[0:64, nsl],
                            selAB[:, 64:128],
                            lrec[p][:, nsl],
                            start=True,
                            stop=True,
                        )
                    nc.vector.tensor_mul(
                        outTh[2 * p][:, jsl], outTh[2 * p][:, jsl], rcP[0:64, :]
                    )
                    nc.vector.tensor_mul(
                        outTh[2 * p + 1][:, jsl], outTh[2 * p + 1][:, jsl], rcA[0:64, :]
                    )

                # ship head outputs: slot r gets our heads for perm-q block r
                # (mirrored to both batch groups; wrong-batch rows of wo are 0)
                for r in range(N_CORES):
                    for h in range(HPC):
                        nc.sync.dma_start(
                            out=a2a_in[j][r, h * DK : (h + 1) * DK, :],
                            in_=outTh[h][
                                :, j * JW + (r % 4) * 256 : j * JW + (r % 4 + 1) * 256
                            ],
                        )
                nc.gpsimd.collective_compute(
                    "AllToAll",
                    mybir.AluOpType.bypass,
                    replica_groups=[list(range(N_CORES))],
                    ins=[a2a_in[j][:]],
                    outs=[a2a_out[j][:]],
                )

            # output projections after all attention: W_O(j0) overlaps A2A(j1)
            for j in range(NJ):
                a2l = []
                for ch in range(16):
                    t = a2l_pool.tile([128, 256], BF16, tag="a2l", name=f"a2l{j}_{ch}")
                    a2l.append(t)
                    nc.sync.dma_start(
                        out=t,
                        in_=a2a_out[j][ch // 2, (ch % 2) * 128 : (ch % 2 + 1) * 128, :],
                    )
                for m in range(2):
                    po = ps.tile([128, 1024], F32, tag="q4", bufs=4)
                    for ch in range(16):
                        for n in range(2):
                            nsl = slice(n * 512, (n + 1) * 512)
                            nc.tensor.matmul(
                                po[:, nsl],
                                a2l[ch][:, m * 128 : (m + 1) * 128],
                                wo_sb[:, ch, nsl],
                                start=(ch == 0),
                                stop=(ch == 15),
                            )
                    ob = osb_pool.tile([128, D], F32, tag="ob")
                    nc.vector.tensor_add(ob, po, bo_rep)
                    nc.sync.dma_start(out=out[j, m * 128 : (m + 1) * 128, :], in_=ob)

    nc.compile()
    return nc


_NC_CACHE = {}


def _get_nc():
    if "nc" not in _NC_CACHE:
        _NC_CACHE["nc"] = _build_nc()
    return _NC_CACHE["nc"]


def kernel(Q, K, V, W_Q, b_Q, W_K, b_K, W_V, b_V, W_O, b_O, _trace=False):
    Q, K, V = (np.asarray(x, np.float32) for x in (Q, K, V))
    W_Q, W_K, W_V, W_O = (np.asarray(x, np.float32) for x in (W_Q, W_K, W_V, W_O))
    b_Q, b_K, b_V, b_O = (np.asarray(x, np.float32) for x in (b_Q, b_K, b_V, b_O))
    scale = np.float32(1.0 / np.sqrt(DK))

    in_maps = []
    for c in range(N_CORES):
        b, g = c // 4, c % 4
        es = slice(g * EC, (g + 1) * EC)
        in_maps.append(
            {
                "xqt": np.ascontiguousarray(Q[b].T[:, _PERM]).astype(ml_dtypes.bfloat16),
                "xkt": np.ascontiguousarray(K[b].T).astype(ml_dtypes.bfloat16),
                "xvt": np.ascontiguousarray(V[b].T).astype(ml_dtypes.bfloat16),
                "wq": _wlayout(W_Q[:, es] * scale).astype(ml_dtypes.bfloat16),
                "wk": _wlayout(W_K[:, es]).astype(ml_dtypes.bfloat16),
                "wv": _wlayout(W_V[:, es]).astype(ml_dtypes.bfloat16),
                "wo": _wo_stack(W_O, b),
                "bq": np.ascontiguousarray(b_Q[es] * scale),
                "bk": np.ascontiguousarray(b_K[es]),
                "bv": np.ascontiguousarray(b_V[es]),
                "bo": b_O,
                "selc": _SELC,
            }
        )

    nc = _get_nc()
    res = run_bass_kernel_spmd(nc, in_maps, list(range(N_CORES)), trace=_trace)

    full = np.empty((B, S, D), np.float32)
    for c in range(N_CORES):
        b, r = c // 4, c % 4
        chunks = res.results[c]["out"]  # [NJ, 256, D]
        full[b, r * 512 : r * 512 + 256, :] = chunks[0]
        full[b, r * 512 + 256 : (r + 1) * 512, :] = chunks[1]
    if _trace:
        return full, res
    return full
